# revision 1
# baseline (speedup 1.0000x reference)
"""CapsLayer2D Trainium2 kernel (8-core SPMD, data-parallel over batch).

Math: per position p (of B*R*C) and capsule n:
  U[n,i,o] = sum_e x[p,i,e] * W[n,i,e,o]          (u_hat)
  b0 = 1/64; 2x { v = squash(sum_i b*U); b += sum_o U*v }; out = squash(sum_i b*U)

Mapping:
  - 8 cores, 2 batches each -> 392 positions/core, 4 pos-blocks of 98.
  - Phase 1: S[p,n,o] = sum_{i,e} x*W as dense K=1024 accumulating matmuls
    (v0 = squash(S/64) since b0 is uniform).
  - Phase 2: per (block, n-pair) unit, u_hat materialized into PSUM via
    block-diagonal-W matmuls (stationary = xT chunk, moving = BD(W), N=256),
    then routing iterations as DVE mul + segmented-reduce ops reading PSUM.
  - Host pre-builds xT (transposed inputs), BD(W), dense W.
"""
import numpy as np

import concourse.bacc as bacc
import concourse.bass as bass
import concourse.mybir as mybir
import concourse.tile as tile
from concourse.bass_utils import run_bass_kernel_spmd

N_CORES = 8
B, R, C = 16, 14, 14
N_IN, D_IN = 64, 16          # i, e
N_CAPS, CAPS_DIM = 10, 16    # n, o
IE = N_IN * D_IN             # 1024
POS = (B // N_CORES) * R * C # 392 positions per core
BLK = 98                     # pos-block size
NBLK = POS // BLK            # 4
NF = N_CAPS // 2             # 5 units of 2 capsules
NCH = IE // 128              # 8 contraction chunks
F32 = mybir.dt.float32

# u_hat matmuls run in bf16 (1 col/cycle at any N; fp32 is 4x slower,
# fp32r needs producer-side rounding the DMA can't provide).
BF16 = mybir.dt.bfloat16


def _squash(nc, pool, s_ap, v_ap, n):
    """v = squash(s): s_ap/v_ap are [98, n, 16] APs; n capsules."""
    P = s_ap.shape[0]
    sq = pool.tile([P, n * 16], F32, tag="sq")
    nc.scalar.activation(sq[:].rearrange("p (n o) -> p n o", o=16), s_ap,
                         mybir.ActivationFunctionType.Square)
    q = pool.tile([P, n], F32, tag="q")
    nc.vector.tensor_reduce(q[:], sq[:].rearrange("p (n o) -> p n o", o=16),
                            axis=mybir.AxisListType.X, op=mybir.AluOpType.add)
    rt = pool.tile([P, n], F32, tag="rt")
    nc.scalar.activation(rt[:], q[:], mybir.ActivationFunctionType.Sqrt)
    qp = pool.tile([P, n], F32, tag="qp")
    nc.vector.tensor_scalar_add(qp[:], q[:], 1.0)
    rc = pool.tile([P, n], F32, tag="rc")
    nc.vector.reciprocal(rc[:], qp[:])
    al = pool.tile([P, n], F32, tag="al")
    nc.vector.tensor_mul(al[:], rt[:], rc[:])
    alb = al[:].unsqueeze(2).broadcast_to([P, n, 16])
    nc.vector.tensor_mul(v_ap, s_ap, alb)


def build_kernel(dbg=False, repeat=1):
    nc = bacc.Bacc("TRN2", target_bir_lowering=False, debug=False,
                   num_devices=N_CORES)
    xT = nc.dram_tensor("xT", [IE, POS], F32, kind="ExternalInput").ap()
    bdw = nc.dram_tensor("bdw", [128, NCH * N_CAPS * 128], BF16,
                         kind="ExternalInput").ap()
    wd = nc.dram_tensor("wd", [IE, N_CAPS * 16], F32, kind="ExternalInput").ap()
    out = nc.dram_tensor("out", [POS, N_CAPS * 16], F32,
                         kind="ExternalOutput").ap()
    if dbg:
        dbg_s0 = nc.dram_tensor("dbg_s0", [BLK, NBLK * 160], F32,
                                kind="ExternalOutput").ap()
        dbg_v0 = nc.dram_tensor("dbg_v0", [BLK, NBLK * 160], F32,
                                kind="ExternalOutput").ap()
        dbg_u = nc.dram_tensor("dbg_u", [BLK, 2048], F32,
                               kind="ExternalOutput").ap()
        dbg_b1 = nc.dram_tensor("dbg_b1", [BLK, 128], F32,
                                kind="ExternalOutput").ap()

    with tile.TileContext(nc) as tc:
        for _rep in range(repeat):
            with tc.tile_pool(name="const", bufs=1) as const, \
                 tc.tile_pool(name="work", bufs=3) as work:
                bdw_t = const.tile([128, NCH * N_CAPS * 128], BF16)
                nc.sync.dma_start(bdw_t[:], bdw[:])
                xtb_t = const.tile([128, NCH * POS], BF16)   # bf16 xT for u_hat
                s0_t = const.tile([BLK, NBLK * 160], F32)    # S/64 per block
                v0_t = const.tile([BLK, NBLK * 160], F32)
                out_t = const.tile([BLK, NBLK * 160], F32)

                # ---- phase 1: S = sum_ie x*W ; v0 = squash(S/64) ----
                with tc.tile_pool(name="p1", bufs=1) as p1, \
                     tc.tile_pool(name="psum_s", bufs=4, space="PSUM") as psum_s:
                    xt_t = p1.tile([128, NCH * POS], F32)    # chunk g at g*POS
                    for g in range(NCH):
                        nc.sync.dma_start(xt_t[:, g * POS:(g + 1) * POS],
                                          xT[g * 128:(g + 1) * 128, :])
                    wd_t = p1.tile([128, NCH * N_CAPS * 16], F32)
                    for g in range(NCH):
                        nc.sync.dma_start(wd_t[:, g * 160:(g + 1) * 160],
                                          wd[g * 128:(g + 1) * 128, :])
                    nc.vector.tensor_copy(xtb_t[:], xt_t[:])
                    for b in range(NBLK):
                        for f in range(NF):
                            ps = psum_s.tile([BLK, 32], F32, tag="ps")
                            for g in range(NCH):
                                nc.tensor.matmul(
                                    ps[:],
                                    xt_t[:, g * POS + b * BLK: g * POS + (b + 1) * BLK],
                                    wd_t[:, g * 160 + f * 32: g * 160 + (f + 1) * 32],
                                    start=(g == 0), stop=(g == NCH - 1))
                            nc.scalar.activation(
                                s0_t[:, b * 160 + f * 32: b * 160 + (f + 1) * 32],
                                ps[:], mybir.ActivationFunctionType.Copy,
                                scale=1.0 / N_IN)
                    for b in range(NBLK):
                        sb = s0_t[:, b * 160:(b + 1) * 160].rearrange(
                            "p (n o) -> p n o", o=16)
                        vb = v0_t[:, b * 160:(b + 1) * 160].rearrange(
                            "p (n o) -> p n o", o=16)
                        _squash(nc, work, sb, vb, N_CAPS)

                # ---- phase 2: u_hat + 2 routing iterations, batched per block --
                # Unit (b,f) u_hat -> PSUM [p,(gi,n2,o)] (gi=8g+i8=i), ACT-drains
                # to bf16 SBUF ub[f*2048:]. Routing per block (5 units at once):
                #   it0: b1 = 1/64 + sum_o U*v0 ; v1 = squash(sum_i b1*U)
                #   it1: b2 = b1 + sum_o U*v1  ; out = squash(sum_i b2*U)
                # Products are bf16 DVE muls (2x mode); contractions are halving
                # add-trees (bf16 2x on wide levels, f32 tail) - ~2x faster than
                # tensor_reduce which has no 2x mode.
                with tc.tile_pool(name="ubp", bufs=2) as ubp, \
                     tc.tile_pool(name="big", bufs=1) as big, \
                     tc.tile_pool(name="psum_u", bufs=2, space="PSUM") as psum_u:
                    for b in range(NBLK):
                        ub = ubp.tile([BLK, NF * 2048], BF16, tag="ub")
                        for f in range(NF):
                            up = psum_u.tile([BLK, 2048], F32, tag="up")
                            for g in range(NCH):
                                lhs = xtb_t[:, g * POS + b * BLK: g * POS + (b + 1) * BLK]
                                rhs = bdw_t[:, g * 1280:(g + 1) * 1280] \
                                    .rearrange("p (i c) -> p i c", c=160) \
                                    [:, :, f * 32:(f + 1) * 32]
                                nc.tensor.matmul(
                                    up[:, g * 256:(g + 1) * 256], lhs, rhs,
                                    start=True, stop=True)
                            nc.scalar.activation(ub[:, f * 2048:(f + 1) * 2048],
                                                 up[:],
                                                 mybir.ActivationFunctionType.Copy)
                            if dbg and b == 0 and f == 0:
                                ucp = work.tile([BLK, 2048], F32, tag="ucp")
                                nc.vector.tensor_copy(ucp[:], up[:])
                                nc.sync.dma_start(dbg_u[:], ucp[:])

                        bco = work.tile([BLK, NF * 128], F32, tag="bco")  # (f,gi,n2)
                        nc.vector.memset(bco[:], 1.0 / N_IN)
                        vb16 = work.tile([BLK, 160], BF16, tag="vb16")
                        nc.vector.tensor_copy(vb16[:],
                                              v0_t[:, b * 160:(b + 1) * 160])
                        Ub = ub[:].rearrange("p (f gi no) -> p f gi no",
                                             f=NF, gi=64, no=32)
                        for it in range(2):
                            # agreement: bco += sum_o U*v (tree over o=16)
                            P = big.tile([BLK, NF * 2048], BF16, tag="P")
                            vbb = vb16[:].rearrange("p (f no) -> p f no", no=32) \
                                .unsqueeze(2).broadcast_to([BLK, NF, 64, 32])
                            nc.vector.tensor_mul(
                                P[:].rearrange("p (f gi no) -> p f gi no",
                                               f=NF, gi=64, no=32), Ub, vbb)
                            with nc.allow_low_precision("bf16 tree sums"):
                                Pv = P[:].rearrange("p (s o) -> p s o", o=16)
                                t1 = big.tile([BLK, NF * 1024], BF16, tag="t1")
                                t1v = t1[:].rearrange("p (s o) -> p s o", o=8)
                                nc.vector.tensor_add(t1v, Pv[:, :, 0:8], Pv[:, :, 8:16])
                                t2 = big.tile([BLK, NF * 512], BF16, tag="t2")
                                t2v = t2[:].rearrange("p (s o) -> p s o", o=4)
                                nc.vector.tensor_add(t2v, t1v[:, :, 0:4], t1v[:, :, 4:8])
                                t3 = big.tile([BLK, NF * 256], BF16, tag="t3")
                                t3v = t3[:].rearrange("p (s o) -> p s o", o=2)
                                nc.vector.tensor_add(t3v, t2v[:, :, 0:2], t2v[:, :, 2:4])
                                agr = work.tile([BLK, NF * 128], F32, tag="agr")
                                nc.vector.tensor_add(
                                    agr[:].rearrange("p (s o) -> p s o", o=1),
                                    t3v[:, :, 0:1], t3v[:, :, 1:2])
                            nc.vector.tensor_add(bco[:], bco[:], agr[:])
                            if dbg and b == 0 and it == 0:
                                nc.sync.dma_start(dbg_b1[:], bco[:, 0:128])
                            # v-sum: s = sum_gi b*U (Q in (f,n,o,gi); tree over gi)
                            bcb = work.tile([BLK, NF * 128], BF16, tag="bcb")
                            nc.vector.tensor_copy(bcb[:], bco[:])
                            Q = big.tile([BLK, NF * 2048], BF16, tag="Q")
                            for f in range(NF):
                                Uq = ub[:, f * 2048:(f + 1) * 2048].rearrange(
                                    "p (gi n o) -> p n o gi", gi=64, n=2, o=16)
                                bbf = bcb[:, f * 128:(f + 1) * 128].rearrange(
                                    "p (gi n) -> p n gi", n=2) \
                                    .unsqueeze(2).broadcast_to([BLK, 2, 16, 64])
                                nc.vector.tensor_mul(
                                    Q[:, f * 2048:(f + 1) * 2048].rearrange(
                                        "p (n o gi) -> p n o gi", n=2, o=16),
                                    Uq, bbf)
                            with nc.allow_low_precision("bf16 tree sums"):
                                Qv = Q[:].rearrange("p (s g) -> p s g", g=64)
                                q1 = big.tile([BLK, NF * 1024], BF16, tag="q1")
                                q1v = q1[:].rearrange("p (s g) -> p s g", g=32)
                                nc.vector.tensor_add(q1v, Qv[:, :, 0:32], Qv[:, :, 32:64])
                                q2 = big.tile([BLK, NF * 512], BF16, tag="q2")
                                q2v = q2[:].rearrange("p (s g) -> p s g", g=16)
                                nc.vector.tensor_add(q2v, q1v[:, :, 0:16], q1v[:, :, 16:32])
                                q3 = big.tile([BLK, NF * 256], BF16, tag="q3")
                                q3v = q3[:].rearrange("p (s g) -> p s g", g=8)
                                nc.vector.tensor_add(q3v, q2v[:, :, 0:8], q2v[:, :, 8:16])
                                q4 = big.tile([BLK, NF * 128], BF16, tag="q4")
                                q4v = q4[:].rearrange("p (s g) -> p s g", g=4)
                                nc.vector.tensor_add(q4v, q3v[:, :, 0:4], q3v[:, :, 4:8])
                                q5 = big.tile([BLK, NF * 64], BF16, tag="q5")
                                q5v = q5[:].rearrange("p (s g) -> p s g", g=2)
                                nc.vector.tensor_add(q5v, q4v[:, :, 0:2], q4v[:, :, 2:4])
                                s_blk = work.tile([BLK, 160], F32, tag="s_blk")
                                nc.vector.tensor_add(
                                    s_blk[:].rearrange("p (s g) -> p s g", g=1),
                                    q5v[:, :, 0:1], q5v[:, :, 1:2])
                            # squash (batched over the block's 10 capsules)
                            if it == 0:
                                v_blk = work.tile([BLK, 160], F32, tag="v_blk")
                                _squash(nc, work,
                                        s_blk[:].rearrange("p (n o) -> p n o", o=16),
                                        v_blk[:].rearrange("p (n o) -> p n o", o=16),
                                        N_CAPS)
                                nc.vector.tensor_copy(vb16[:], v_blk[:])
                            else:
                                dst = out_t[:, b * 160:(b + 1) * 160]
                                _squash(nc, work,
                                        s_blk[:].rearrange("p (n o) -> p n o", o=16),
                                        dst.rearrange("p (n o) -> p n o", o=16),
                                        N_CAPS)

                for b in range(NBLK):
                    nc.sync.dma_start(out[b * BLK:(b + 1) * BLK, :],
                                      out_t[:, b * 160:(b + 1) * 160])
                if dbg:
                    nc.sync.dma_start(dbg_s0[:], s0_t[:])
                    nc.sync.dma_start(dbg_v0[:], v0_t[:])
    nc.compile()
    return nc


def _host_prep(inputs, W):
    """Build per-core input maps from full inputs."""
    x = np.ascontiguousarray(inputs, dtype=np.float32).reshape(B, R * C, IE)
    Wf = np.ascontiguousarray(W, dtype=np.float32)  # [n, i, e, o]
    # bdw[(i8,e), (g,n,i8,o)]
    Wg = Wf.reshape(N_CAPS, 8, 8, D_IN, CAPS_DIM)   # [n, g, i8, e, o]
    bdw6 = np.zeros((8, D_IN, 8, 8, N_CAPS, CAPS_DIM), dtype=np.float32)
    for i8 in range(8):
        # [n, g, e, o] -> [e, g, n, o]
        bdw6[i8, :, :, i8, :, :] = Wg[:, :, i8, :, :].transpose(2, 1, 0, 3)
    import ml_dtypes
    bdw = bdw6.reshape(128, NCH * N_CAPS * 128).astype(ml_dtypes.bfloat16)
    wd = Wf.transpose(1, 2, 0, 3).reshape(IE, N_CAPS * CAPS_DIM)
    bpc = B // N_CORES
    in_maps = []
    for c in range(N_CORES):
        xc = x[c * bpc:(c + 1) * bpc].reshape(POS, IE)
        in_maps.append({
            "xT": np.ascontiguousarray(xc.T),
            "bdw": bdw,
            "wd": wd,
        })
    return in_maps


_NC_CACHE = []


def kernel(inputs: np.ndarray, W: np.ndarray) -> np.ndarray:
    in_maps = _host_prep(inputs, W)
    if not _NC_CACHE:
        _NC_CACHE.append(build_kernel())
    nc = _NC_CACHE[0]
    res = run_bass_kernel_spmd(nc, in_maps, list(range(N_CORES)))
    outs = [res.results[c]["out"] for c in range(N_CORES)]
    full = np.concatenate(outs, axis=0)  # [3136, 160]
    return full.reshape(B, R, C, N_CAPS, CAPS_DIM)



# revision 7
# speedup vs baseline: 1.0400x; 1.0400x over previous
"""CapsLayer2D Trainium2 kernel (8-core SPMD, data-parallel over batch).

Math: per position p (of B*R*C) and capsule n:
  U[n,i,o] = sum_e x[p,i,e] * W[n,i,e,o]          (u_hat)
  b0 = 1/64; 2x { v = squash(sum_i b*U); b += sum_o U*v }; out = squash(sum_i b*U)

Since b = 1/64 + A (A = accumulated agreement), sum_i b*U = s0 + sum_i A*U
with s0 = (1/64) sum_i U from phase 1 - so routing tracks A only (no memset,
A kept in bf16).

Mapping:
  - 8 cores, 2 batches each -> 392 positions/core, 4 pos-blocks of 98.
  - Phase 1: S[p,n,o] = sum_{i,e} x*W as dense K=1024 accumulating matmuls
    (v0 = squash(S/64) since b0 is uniform).
  - Phase 2: per (block, n-pair) unit, u_hat materialized into PSUM via
    block-diagonal-W matmuls (stationary = xT chunk, moving = BD(W), N=256),
    then routing iterations as elementwise mul + segmented-reduce add-trees.
  - u_hat columns are laid out (g, i8, o, n2) [NOT (..., n2, o)]: with the
    capsule-pair index n2 innermost, every operand of every big mul/tree is
    last-dim-contiguous bf16 (broadcasts over v / A land on middle dims), so
    all DVE TensorTensor ops hit the 2x perf mode.
  - The mul/tree work of capsule-pair unit f=4 runs on the Pool engine
    (gpsimd) in parallel with f=0..3 on DVE, roughly balancing the two.
  - Host pre-builds xT (transposed inputs), BD(W), dense W.
"""
import numpy as np

import concourse.bacc as bacc
import concourse.bass as bass
import concourse.mybir as mybir
import concourse.tile as tile
from concourse.bass_utils import run_bass_kernel_spmd

N_CORES = 8
B, R, C = 16, 14, 14
N_IN, D_IN = 64, 16          # i, e
N_CAPS, CAPS_DIM = 10, 16    # n, o
IE = N_IN * D_IN             # 1024
POS = (B // N_CORES) * R * C # 392 positions per core
BLK = 98                     # pos-block size
NBLK = POS // BLK            # 4
NF = N_CAPS // 2             # 5 units of 2 capsules
NCH = IE // 128              # 8 contraction chunks
F32 = mybir.dt.float32

# u_hat matmuls run in bf16 (1 col/cycle at any N; fp32 is 4x slower,
# fp32r needs producer-side rounding the DMA can't provide).
BF16 = mybir.dt.bfloat16

# DVE handles capsule-pair units f < FD, Pool (gpsimd) handles f >= FD.
FD = 4


def _squash_no(nc, pool, s_ap, v_ap, n):
    """v = squash(s) for (n, o)-ordered APs [P, n, 16] (phase-1 layout)."""
    P = s_ap.shape[0]
    sq = pool.tile([P, n * 16], F32, tag="sq")
    nc.scalar.activation(sq[:].rearrange("p (n o) -> p n o", o=16), s_ap,
                         mybir.ActivationFunctionType.Square)
    q = pool.tile([P, n], F32, tag="q")
    nc.vector.tensor_reduce(q[:], sq[:].rearrange("p (n o) -> p n o", o=16),
                            axis=mybir.AxisListType.X, op=mybir.AluOpType.add)
    rt = pool.tile([P, n], F32, tag="rt")
    nc.scalar.activation(rt[:], q[:], mybir.ActivationFunctionType.Sqrt)
    qp = pool.tile([P, n], F32, tag="qp")
    nc.vector.tensor_scalar_add(qp[:], q[:], 1.0)
    rc = pool.tile([P, n], F32, tag="rc")
    nc.vector.reciprocal(rc[:], qp[:])
    al = pool.tile([P, n], F32, tag="al")
    nc.vector.tensor_mul(al[:], rt[:], rc[:])
    alb = al[:].unsqueeze(2).broadcast_to([P, n, 16])
    nc.vector.tensor_mul(v_ap, s_ap, alb)


def _squash_on(nc, pool, s_t, v_ap):
    """v = squash(s) for an (f, o, n2)-ordered s tile [P, 160] (routing
    layout). v_ap must be a [P, NF, 16, 2]-shaped AP in op dims (f, o, n2).
    """
    P = s_t.shape[0]
    s_fon = s_t[:].rearrange("p (f o n) -> p f o n", o=16, n=2)
    sq = pool.tile([P, 160], F32, tag="sq")
    nc.scalar.activation(sq[:].rearrange("p (f o n) -> p f o n", o=16, n=2),
                         s_fon, mybir.ActivationFunctionType.Square)
    # reduce over o (middle in storage): 4D view [P, f, n2, o], axis=X
    q = pool.tile([P, N_CAPS], F32, tag="q")
    nc.vector.tensor_reduce(q[:].rearrange("p (f n) -> p f n", n=2),
                            sq[:].rearrange("p (f o n) -> p f n o",
                                            o=16, n=2),
                            axis=mybir.AxisListType.X, op=mybir.AluOpType.add)
    rt = pool.tile([P, N_CAPS], F32, tag="rt")
    nc.scalar.activation(rt[:], q[:], mybir.ActivationFunctionType.Sqrt)
    qp = pool.tile([P, N_CAPS], F32, tag="qp")
    nc.vector.tensor_scalar_add(qp[:], q[:], 1.0)
    rc = pool.tile([P, N_CAPS], F32, tag="rc")
    nc.vector.reciprocal(rc[:], qp[:])
    al = pool.tile([P, N_CAPS], F32, tag="al")
    nc.vector.tensor_mul(al[:], rt[:], rc[:])
    # al is [P, (f, n2)]; broadcast over o (middle dim of (f, o, n2))
    alb = al[:].rearrange("p (f n) -> p f n", n=2) \
        .unsqueeze(2).broadcast_to([P, NF, 16, 2])
    nc.vector.tensor_mul(v_ap, s_fon, alb)


def build_kernel(dbg=False, repeat=1):
    nc = bacc.Bacc("TRN2", target_bir_lowering=False, debug=False,
                   num_devices=N_CORES)
    xT = nc.dram_tensor("xT", [IE, POS], F32, kind="ExternalInput").ap()
    bdw = nc.dram_tensor("bdw", [128, NCH * N_CAPS * 128], BF16,
                         kind="ExternalInput").ap()
    wd = nc.dram_tensor("wd", [IE, N_CAPS * 16], F32, kind="ExternalInput").ap()
    out = nc.dram_tensor("out", [POS, N_CAPS * 16], F32,
                         kind="ExternalOutput").ap()

    with tile.TileContext(nc) as tc:
        for _rep in range(repeat):
            with tc.tile_pool(name="const", bufs=1) as const, \
                 tc.tile_pool(name="work", bufs=3) as work:
                bdw_t = const.tile([128, NCH * N_CAPS * 128], BF16)
                nc.sync.dma_start(bdw_t[:], bdw[:])
                xtb_t = const.tile([128, NCH * POS], BF16)   # bf16 xT for u_hat
                s0_t = const.tile([BLK, NBLK * 160], F32)    # S/64 per block
                v0_t = const.tile([BLK, NBLK * 160], F32)
                out_t = const.tile([BLK, NBLK * 160], F32)

                # ---- phase 1: S = sum_ie x*W ; v0 = squash(S/64) ----
                with tc.tile_pool(name="p1", bufs=1) as p1, \
                     tc.tile_pool(name="psum_s", bufs=4, space="PSUM") as psum_s:
                    xt_t = p1.tile([128, NCH * POS], F32)    # chunk g at g*POS
                    for g in range(NCH):
                        nc.sync.dma_start(xt_t[:, g * POS:(g + 1) * POS],
                                          xT[g * 128:(g + 1) * 128, :])
                    wd_t = p1.tile([128, NCH * N_CAPS * 16], F32)
                    for g in range(NCH):
                        nc.sync.dma_start(wd_t[:, g * 160:(g + 1) * 160],
                                          wd[g * 128:(g + 1) * 128, :])
                    nc.vector.tensor_copy(xtb_t[:], xt_t[:])
                    for b in range(NBLK):
                        for f in range(NF):
                            ps = psum_s.tile([BLK, 32], F32, tag="ps")
                            for g in range(NCH):
                                nc.tensor.matmul(
                                    ps[:],
                                    xt_t[:, g * POS + b * BLK: g * POS + (b + 1) * BLK],
                                    wd_t[:, g * 160 + f * 32: g * 160 + (f + 1) * 32],
                                    start=(g == 0), stop=(g == NCH - 1))
                            nc.scalar.activation(
                                s0_t[:, b * 160 + f * 32: b * 160 + (f + 1) * 32],
                                ps[:], mybir.ActivationFunctionType.Copy,
                                scale=1.0 / N_IN)
                    for b in range(NBLK):
                        sb = s0_t[:, b * 160:(b + 1) * 160].rearrange(
                            "p (n o) -> p n o", o=16)
                        vb = v0_t[:, b * 160:(b + 1) * 160].rearrange(
                            "p (n o) -> p n o", o=16)
                        _squash_no(nc, work, sb, vb, N_CAPS)

                # ---- phase 2: u_hat + 2 routing iterations, batched per block --
                # Unit (b,f) u_hat -> PSUM [p,(g,i8,o,n2)] , ACT-drains to bf16
                # SBUF ub[f*2048:]. Per-f layout (gi, o, n2), n2 innermost.
                # Routing per block:
                #   it0: A = sum_o U*v0          ; v1 = squash(s0 + sum_gi A*U)
                #   it1: A += sum_o U*v1         ; out = squash(s0 + sum_gi A*U)
                # All products / tree levels are bf16 with last-dim-contiguous
                # APs -> DVE 2x mode. Unit f=4 runs on Pool (gpsimd), f=0..3
                # on DVE.
                def split(lo_hi_op):
                    """Issue op on DVE for f<FD slice and Pool for the rest."""
                    lo_hi_op(nc.vector, 0, FD)
                    lo_hi_op(nc.gpsimd, FD, NF)

                with tc.tile_pool(name="ubp", bufs=2) as ubp, \
                     tc.tile_pool(name="big", bufs=1) as big, \
                     tc.tile_pool(name="psum_u", bufs=2, space="PSUM") as psum_u:
                    for b in range(NBLK):
                        ub = ubp.tile([BLK, NF * 2048], BF16, tag="ub")
                        for f in range(NF):
                            up = psum_u.tile([BLK, 2048], F32, tag="up")
                            for g in range(NCH):
                                lhs = xtb_t[:, g * POS + b * BLK: g * POS + (b + 1) * BLK]
                                rhs = bdw_t[:, g * 1280:(g + 1) * 1280] \
                                    .rearrange("p (i c) -> p i c", c=160) \
                                    [:, :, f * 32:(f + 1) * 32]
                                nc.tensor.matmul(
                                    up[:, g * 256:(g + 1) * 256], lhs, rhs,
                                    start=True, stop=True)
                            nc.scalar.activation(ub[:, f * 2048:(f + 1) * 2048],
                                                 up[:],
                                                 mybir.ActivationFunctionType.Copy)

                        A = work.tile([BLK, NF * 128], BF16, tag="A")  # (f,gi,n2)
                        vb16 = work.tile([BLK, 160], BF16, tag="vb16")  # (f,o,n2)
                        nc.vector.tensor_copy(
                            vb16[:].rearrange("p (f o n) -> p f n o", o=16, n=2),
                            v0_t[:, b * 160:(b + 1) * 160].rearrange(
                                "p (f n o) -> p f n o", f=NF, n=2, o=16))
                        # op-dim views (f, gi, o/n2 structure); per-f slices cut
                        # on the f axis.
                        Ub = ub[:].rearrange("p (f gi on) -> p f gi on",
                                             f=NF, gi=64, on=32)
                        for it in range(2):
                            # agreement: A (+)= sum_o U*v  (tree over o=16)
                            P = big.tile([BLK, NF * 2048], BF16, tag="P")
                            Pv = P[:].rearrange("p (f gi on) -> p f gi on",
                                                f=NF, gi=64, on=32)
                            vbb = vb16[:].rearrange("p (f on) -> p f on", on=32) \
                                .unsqueeze(2).broadcast_to([BLK, NF, 64, 32])
                            split(lambda e, lo, hi: e.tensor_mul(
                                Pv[:, lo:hi], Ub[:, lo:hi], vbb[:, lo:hi]))
                            with nc.allow_low_precision("bf16 tree sums"):
                                # levels over o; storage (f, gi, o, n2)
                                Po = P[:].rearrange("p (s o n) -> p s o n",
                                                    o=16, n=2)
                                t1 = big.tile([BLK, NF * 1024], BF16, tag="t1")
                                t1v = t1[:].rearrange("p (s o n) -> p s o n",
                                                      o=8, n=2)
                                split(lambda e, lo, hi: e.tensor_add(
                                    t1v[:, lo * 64:hi * 64],
                                    Po[:, lo * 64:hi * 64, 0:8],
                                    Po[:, lo * 64:hi * 64, 8:16]))
                                t2 = big.tile([BLK, NF * 512], BF16, tag="t2")
                                t2v = t2[:].rearrange("p (s o n) -> p s o n",
                                                      o=4, n=2)
                                split(lambda e, lo, hi: e.tensor_add(
                                    t2v[:, lo * 64:hi * 64],
                                    t1v[:, lo * 64:hi * 64, 0:4],
                                    t1v[:, lo * 64:hi * 64, 4:8]))
                                t3 = big.tile([BLK, NF * 256], BF16, tag="t3")
                                t3v = t3[:].rearrange("p (s o n) -> p s o n",
                                                      o=2, n=2)
                                split(lambda e, lo, hi: e.tensor_add(
                                    t3v[:, lo * 64:hi * 64],
                                    t2v[:, lo * 64:hi * 64, 0:2],
                                    t2v[:, lo * 64:hi * 64, 2:4]))
                                Av = A[:].rearrange("p (s o n) -> p s o n",
                                                    o=1, n=2)
                                if it == 0:
                                    split(lambda e, lo, hi: e.tensor_add(
                                        Av[:, lo * 64:hi * 64],
                                        t3v[:, lo * 64:hi * 64, 0:1],
                                        t3v[:, lo * 64:hi * 64, 1:2]))
                                else:
                                    agr = work.tile([BLK, NF * 128], BF16,
                                                    tag="agr")
                                    agv = agr[:].rearrange(
                                        "p (s o n) -> p s o n", o=1, n=2)
                                    split(lambda e, lo, hi: e.tensor_add(
                                        agv[:, lo * 64:hi * 64],
                                        t3v[:, lo * 64:hi * 64, 0:1],
                                        t3v[:, lo * 64:hi * 64, 1:2]))
                                    Af = A[:].rearrange("p (f gn) -> p f gn",
                                                        gn=128)
                                    agf = agr[:].rearrange("p (f gn) -> p f gn",
                                                           gn=128)
                                    split(lambda e, lo, hi: e.tensor_add(
                                        Af[:, lo:hi], Af[:, lo:hi],
                                        agf[:, lo:hi]))
                            # v-sum: s = s0 + sum_gi A*U  (tree over gi=64)
                            # op dims (s=(f,gi), o, n2): broadcast of A over o
                            # sits mid-AP, n2 stays contiguous-last -> 2x.
                            Q = big.tile([BLK, NF * 2048], BF16, tag="Q")
                            Qv = Q[:].rearrange("p (s o n) -> p s o n",
                                                o=16, n=2)
                            Uq = ub[:].rearrange("p (s o n) -> p s o n",
                                                 o=16, n=2)
                            Ab = A[:].rearrange("p (s n) -> p s n", n=2) \
                                .unsqueeze(2).broadcast_to([BLK, NF * 64, 16, 2])
                            split(lambda e, lo, hi: e.tensor_mul(
                                Qv[:, lo * 64:hi * 64], Uq[:, lo * 64:hi * 64],
                                Ab[:, lo * 64:hi * 64]))
                            with nc.allow_low_precision("bf16 tree sums"):
                                Qg = Q[:].rearrange("p (f g s) -> p f g s",
                                                    f=NF, g=64, s=32)
                                q1 = big.tile([BLK, NF * 1024], BF16, tag="q1")
                                q1v = q1[:].rearrange("p (f g s) -> p f g s",
                                                      f=NF, g=32, s=32)
                                split(lambda e, lo, hi: e.tensor_add(
                                    q1v[:, lo:hi], Qg[:, lo:hi, 0:32],
                                    Qg[:, lo:hi, 32:64]))
                                q2 = big.tile([BLK, NF * 512], BF16, tag="q2")
                                q2v = q2[:].rearrange("p (f g s) -> p f g s",
                                                      f=NF, g=16, s=32)
                                split(lambda e, lo, hi: e.tensor_add(
                                    q2v[:, lo:hi], q1v[:, lo:hi, 0:16],
                                    q1v[:, lo:hi, 16:32]))
                                q3 = big.tile([BLK, NF * 256], BF16, tag="q3")
                                q3v = q3[:].rearrange("p (f g s) -> p f g s",
                                                      f=NF, g=8, s=32)
                                split(lambda e, lo, hi: e.tensor_add(
                                    q3v[:, lo:hi], q2v[:, lo:hi, 0:8],
                                    q2v[:, lo:hi, 8:16]))
                                q4 = big.tile([BLK, NF * 128], BF16, tag="q4")
                                q4v = q4[:].rearrange("p (f g s) -> p f g s",
                                                      f=NF, g=4, s=32)
                                split(lambda e, lo, hi: e.tensor_add(
                                    q4v[:, lo:hi], q3v[:, lo:hi, 0:4],
                                    q3v[:, lo:hi, 4:8]))
                                q5 = big.tile([BLK, NF * 64], BF16, tag="q5")
                                q5v = q5[:].rearrange("p (f g s) -> p f g s",
                                                      f=NF, g=2, s=32)
                                split(lambda e, lo, hi: e.tensor_add(
                                    q5v[:, lo:hi], q4v[:, lo:hi, 0:2],
                                    q4v[:, lo:hi, 2:4]))
                                s_blk = work.tile([BLK, 160], F32, tag="s_blk")
                                sv = s_blk[:].rearrange("p (f g s) -> p f g s",
                                                        f=NF, g=1, s=32)
                                split(lambda e, lo, hi: e.tensor_add(
                                    sv[:, lo:hi], q5v[:, lo:hi, 0:1],
                                    q5v[:, lo:hi, 1:2]))
                            # + s0 (permuted read: s0 is (f, n2, o)-ordered)
                            s0b = s0_t[:, b * 160:(b + 1) * 160].rearrange(
                                "p (f n o) -> p f o n", n=2, o=16)
                            nc.vector.tensor_add(
                                s_blk[:].rearrange("p (f o n) -> p f o n",
                                                   o=16, n=2),
                                s_blk[:].rearrange("p (f o n) -> p f o n",
                                                   o=16, n=2),
                                s0b)
                            # squash (batched over the block's 10 capsules)
                            if it == 0:
                                v_blk = work.tile([BLK, 160], F32, tag="v_blk")
                                _squash_on(nc, work, s_blk,
                                           v_blk[:].rearrange(
                                               "p (f o n) -> p f o n",
                                               o=16, n=2))
                                nc.vector.tensor_copy(vb16[:], v_blk[:])
                            else:
                                dst = out_t[:, b * 160:(b + 1) * 160]
                                _squash_on(nc, work, s_blk,
                                           dst.rearrange(
                                               "p (f n o) -> p f o n",
                                               n=2, o=16))

                for b in range(NBLK):
                    nc.sync.dma_start(out[b * BLK:(b + 1) * BLK, :],
                                      out_t[:, b * 160:(b + 1) * 160])
    nc.compile()
    return nc


def _host_prep(inputs, W):
    """Build per-core input maps from full inputs."""
    x = np.ascontiguousarray(inputs, dtype=np.float32).reshape(B, R * C, IE)
    Wf = np.ascontiguousarray(W, dtype=np.float32)  # [n, i, e, o]
    # bdw[(i8,e), (g, i8, f, o, n2)]
    Wg = Wf.reshape(N_CAPS, 8, 8, D_IN, CAPS_DIM)   # [n, g, i8, e, o]
    bdw6 = np.zeros((8, D_IN, 8, 8, N_CAPS, CAPS_DIM), dtype=np.float32)
    for i8 in range(8):
        # [n, g, e, o] -> [e, g, n, o]
        bdw6[i8, :, :, i8, :, :] = Wg[:, :, i8, :, :].transpose(2, 1, 0, 3)
    # cols (g, i8, n, o) -> (g, i8, f, o, n2)
    bdw7 = bdw6.reshape(8, D_IN, 8, 8, NF, 2, CAPS_DIM)
    bdw7 = bdw7.transpose(0, 1, 2, 3, 4, 6, 5)
    import ml_dtypes
    bdw_a = np.ascontiguousarray(bdw7).reshape(
        128, NCH * N_CAPS * 128).astype(ml_dtypes.bfloat16)
    wd = Wf.transpose(1, 2, 0, 3).reshape(IE, N_CAPS * CAPS_DIM)
    bpc = B // N_CORES
    in_maps = []
    for c in range(N_CORES):
        xc = x[c * bpc:(c + 1) * bpc].reshape(POS, IE)
        in_maps.append({
            "xT": np.ascontiguousarray(xc.T),
            "bdw": bdw_a,
            "wd": wd,
        })
    return in_maps


_NC_CACHE = []


def kernel(inputs: np.ndarray, W: np.ndarray) -> np.ndarray:
    in_maps = _host_prep(inputs, W)
    if not _NC_CACHE:
        _NC_CACHE.append(build_kernel())
    nc = _NC_CACHE[0]
    res = run_bass_kernel_spmd(nc, in_maps, list(range(N_CORES)))
    outs = [res.results[c]["out"] for c in range(N_CORES)]
    full = np.concatenate(outs, axis=0)  # [3136, 160]
    return full.reshape(B, R, C, N_CAPS, CAPS_DIM)


# revision 35
# speedup vs baseline: 1.3885x; 1.3351x over previous
"""CapsLayer2D Trainium2 kernel (8-core SPMD, data-parallel over batch).

Math: per position p (of B*R*C) and capsule n:
  U[n,i,o] = sum_e x[p,i,e] * W[n,i,e,o]          (u_hat)
  b0 = 1/64; 2x { v = squash(sum_i b*U); b += sum_o U*v }; out = squash(sum_i b*U)

Since b = 1/64 + A (A = accumulated agreement), sum_i b*U = s0 + sum_i A*U
with s0 = (1/64) sum_i U - so routing tracks A only (no memset, A in bf16).

Mapping:
  - 8 cores, 2 batches each -> 392 positions/core, 4 pos-blocks of 98.
  - Per (block, n-pair f) unit: u_hat into PSUM via block-diagonal-W matmuls
    (stationary = bf16 xT chunk, moving = BD(W), N=256) + a dense-W
    accumulating matmul chain for s0 (drained with scale=1/64).
  - Routing iterations as elementwise mul + segmented-reduce add-trees.
  - u_hat columns are laid out (g, i8, o, n2) [NOT (..., n2, o)]: with the
    capsule-pair index n2 innermost, every operand of every big mul/tree is
    last-dim-contiguous bf16 (broadcasts over v / A land on middle dims), so
    all DVE TensorTensor ops hit the 2x perf mode.
  - The mul/tree work of capsule-pair unit f=4 runs on the Pool engine
    (gpsimd) in parallel with f=0..3 on DVE, roughly balancing the two.
  - Host pre-builds bf16 xT (chunked+transposed inputs), BD(W), dense W.
"""
import numpy as np

import concourse.bacc as bacc
import concourse.bass as bass
import concourse.mybir as mybir
import concourse.tile as tile
from concourse.bass_utils import run_bass_kernel_spmd

N_CORES = 8
B, R, C = 16, 14, 14
N_IN, D_IN = 64, 16          # i, e
N_CAPS, CAPS_DIM = 10, 16    # n, o
IE = N_IN * D_IN             # 1024
POS = (B // N_CORES) * R * C # 392 positions per core
BLK = 98                     # pos-block size
NBLK = POS // BLK            # 4
NF = N_CAPS // 2             # 5 units of 2 capsules
NCH = IE // 128              # 8 contraction chunks
F32 = mybir.dt.float32

# u_hat matmuls run in bf16 (1 col/cycle at any N; fp32 is 4x slower,
# fp32r needs producer-side rounding the DMA can't provide).
BF16 = mybir.dt.bfloat16

# DVE handles capsule-pair units f < FD, Pool (gpsimd) handles f >= FD.
FD = 4


def _squash_on(nc, pool, s_t, v_ap, lowp=False):
    """v = squash(s) for an (f, o, n2)-ordered s tile [P, 160] (routing
    layout). v_ap must be a [P, NF, 16, 2]-shaped AP in op dims (f, o, n2).
    Square runs on DVE (cheap f32 mul) to avoid an Act round-trip; only
    Sqrt uses the Act engine.
    """
    P = s_t.shape[0]
    s_fon = s_t[:].rearrange("p (f o n) -> p f o n", o=16, n=2)
    sq = pool.tile([P, 160], F32, tag="sq")
    nc.vector.tensor_mul(sq[:], s_t[:], s_t[:])
    # reduce over o (middle in storage): 4D view [P, f, n2, o], axis=X
    q = pool.tile([P, N_CAPS], F32, tag="q")
    nc.vector.tensor_reduce(q[:].rearrange("p (f n) -> p f n", n=2),
                            sq[:].rearrange("p (f o n) -> p f n o",
                                            o=16, n=2),
                            axis=mybir.AxisListType.X, op=mybir.AluOpType.add)
    rt = pool.tile([P, N_CAPS], F32, tag="rt")
    nc.scalar.activation(rt[:], q[:], mybir.ActivationFunctionType.Sqrt)
    qp = pool.tile([P, N_CAPS], F32, tag="qp")
    nc.vector.tensor_scalar_add(qp[:], q[:], 1.0)
    rc = pool.tile([P, N_CAPS], F32, tag="rc")
    nc.vector.reciprocal(rc[:], qp[:])
    al = pool.tile([P, N_CAPS], F32, tag="al")
    nc.vector.tensor_mul(al[:], rt[:], rc[:])
    # al is [P, (f, n2)]; broadcast over o (middle dim of (f, o, n2))
    alb = al[:].rearrange("p (f n) -> p f n", n=2) \
        .unsqueeze(2).broadcast_to([P, NF, 16, 2])
    if lowp:
        with nc.allow_low_precision("bf16 v"):
            nc.vector.tensor_mul(v_ap, s_fon, alb)
    else:
        nc.vector.tensor_mul(v_ap, s_fon, alb)


def build_kernel(dbg=False, repeat=1):
    nc = bacc.Bacc("TRN2", target_bir_lowering=False, debug=False,
                   num_devices=N_CORES)
    xT = nc.dram_tensor("xT", [D_IN, N_IN * POS], BF16,
                        kind="ExternalInput").ap()
    xF = nc.dram_tensor("xF", [128, NCH * POS], BF16,
                        kind="ExternalInput").ap()
    w8 = nc.dram_tensor("w8", [D_IN, NCH * N_CAPS * 128], BF16,
                        kind="ExternalInput").ap()
    wd = nc.dram_tensor("wd", [128, NCH * N_CAPS * 16], BF16,
                        kind="ExternalInput").ap()
    out = nc.dram_tensor("out", [POS, N_CAPS * 16], F32,
                         kind="ExternalOutput").ap()

    with tile.TileContext(nc) as tc:
        for _rep in range(repeat):
            with tc.tile_pool(name="const", bufs=1) as const, \
                 tc.tile_pool(name="work", bufs=3) as work:
                # per-chunk DMAs, g-ordered, so matmul g can start as soon as
                # its chunk lands. W is sent compact (w8, the 8x-smaller
                # nonzero block of the block-diagonal layout); x is sent with
                # e on partitions so every K=16 matmul sits at base 0.
                # one DMA per tensor (HWDGE costs ~630ns/descriptor), the two
                # s0-path tensors and the two u_hat tensors on separate queues
                w8_t = const.tile([D_IN, NCH * N_CAPS * 128], BF16)
                xtb_t = const.tile([D_IN, N_IN * POS], BF16)
                xf_t = const.tile([128, NCH * POS], BF16)
                wd_t = const.tile([128, NCH * N_CAPS * 16], BF16)
                nc.scalar.dma_start(wd_t[:], wd[:])
                nc.sync.dma_start(xtb_t[:], xT[:])
                nc.scalar.dma_start(xf_t[:], xF[:])
                nc.sync.dma_start(w8_t[:], w8[:])
                out_t = const.tile([BLK, NBLK * 160], F32)

                def split(lo_hi_op):
                    """Issue op on DVE for f<FD slice and Pool for the rest."""
                    lo_hi_op(nc.vector, 0, FD)
                    lo_hi_op(nc.gpsimd, FD, NF)

                with tc.tile_pool(name="ubp", bufs=2) as ubp, \
                     tc.tile_pool(name="psum_s", bufs=2, space="PSUM") as psum_s, \
                     tc.tile_pool(name="big", bufs=1) as big, \
                     tc.tile_pool(name="psum_u", bufs=2, space="PSUM") as psum_u:
                    for b in range(NBLK):
                        # ---- u_hat (PSUM, bf16 matmuls) + s0 (dense-W) ----
                        # g-outer: chunk g's matmuls start as soon as its DMA
                        # lands; one [98,1280] PSUM tile + Act drain per g.
                        ub = ubp.tile([BLK, NF * 2048], BF16, tag="ub")
                        s0_t = work.tile([BLK, 160], F32, tag="s0")
                        ps = psum_s.tile([BLK, 160], F32, tag="ps")
                        ubf = ub[:].rearrange("p (f gc) -> p f gc", f=NF)
                        ub5 = ub[:].rearrange("p (f g i c) -> p f g i c",
                                              f=NF, g=NCH, i=8)
                        for g in range(NCH):
                            # up3 columns (i8, f, o, n2); one K=16 matmul per
                            # input capsule against compact W. s0 accumulates
                            # via a dense K=128 chain (8 matmuls/block).
                            up3 = psum_u.tile([BLK, NF * 256], F32, tag="up3")
                            nc.tensor.matmul(
                                ps[:],
                                xf_t[:, g * POS + b * BLK:
                                     g * POS + (b + 1) * BLK],
                                wd_t[:, g * 160:(g + 1) * 160],
                                start=(g == 0), stop=(g == NCH - 1))
                            for i8 in range(8):
                                lhs = xtb_t[:, (g * 8 + i8) * POS + b * BLK:
                                            (g * 8 + i8) * POS + (b + 1) * BLK]
                                rhs = w8_t[:, (g * 8 + i8) * 160:
                                           (g * 8 + i8 + 1) * 160]
                                # a matmul's PSUM write must not cross a 2KB
                                # bank boundary: split i8=3 (@1920B) / 6
                                # (@3840B) regions at the boundary.
                                cut = {3: 32, 6: 64}.get(i8)
                                if cut is None:
                                    nc.tensor.matmul(
                                        up3[:, i8 * 160:(i8 + 1) * 160],
                                        lhs, rhs, start=True, stop=True)
                                else:
                                    nc.tensor.matmul(
                                        up3[:, i8 * 160:i8 * 160 + cut],
                                        lhs, rhs[:, 0:cut],
                                        start=True, stop=True)
                                    nc.tensor.matmul(
                                        up3[:, i8 * 160 + cut:(i8 + 1) * 160],
                                        lhs, rhs[:, cut:160],
                                        start=True, stop=True)
                            nc.scalar.activation(
                                ub5[:, :, g],
                                up3[:].rearrange("p (i f c) -> p f i c",
                                                 i=8, c=32),
                                mybir.ActivationFunctionType.Copy)
                        nc.scalar.activation(
                            s0_t[:], ps[:],
                            mybir.ActivationFunctionType.Copy,
                            scale=1.0 / N_IN)

                        # ---- v0 = squash(s0), straight into bf16 vb16 ----
                        A = work.tile([BLK, NF * 128], BF16, tag="A")  # (f,gi,n2)
                        vb16 = work.tile([BLK, 160], BF16, tag="vb16")  # (f,o,n2)
                        _squash_on(nc, work, s0_t,
                                   vb16[:].rearrange("p (f o n) -> p f o n",
                                                     o=16, n=2), lowp=True)
                        # op-dim views; per-f slices cut on the f / s axes.
                        Ub = ub[:].rearrange("p (f gi on) -> p f gi on",
                                             f=NF, gi=64, on=32)
                        for it in range(2):
                            # agreement: A (+)= sum_o U*v  (tree over o=16)
                            P = big.tile([BLK, NF * 2048], BF16, tag="PQ")
                            Pv = P[:].rearrange("p (f gi on) -> p f gi on",
                                                f=NF, gi=64, on=32)
                            vbb = vb16[:].rearrange("p (f on) -> p f on", on=32) \
                                .unsqueeze(2).broadcast_to([BLK, NF, 64, 32])
                            split(lambda e, lo, hi: e.tensor_mul(
                                Pv[:, lo:hi], Ub[:, lo:hi], vbb[:, lo:hi]))
                            with nc.allow_low_precision("bf16 tree sums"):
                                # levels over o; storage (f, gi, o, n2)
                                Po = P[:].rearrange("p (s o n) -> p s o n",
                                                    o=16, n=2)
                                t1 = big.tile([BLK, NF * 1024], BF16, tag="tq1")
                                t1v = t1[:].rearrange("p (s o n) -> p s o n",
                                                      o=8, n=2)
                                split(lambda e, lo, hi: e.tensor_add(
                                    t1v[:, lo * 64:hi * 64],
                                    Po[:, lo * 64:hi * 64, 0:8],
                                    Po[:, lo * 64:hi * 64, 8:16]))
                                t2 = big.tile([BLK, NF * 512], BF16, tag="tq2")
                                t2v = t2[:].rearrange("p (s o n) -> p s o n",
                                                      o=4, n=2)
                                split(lambda e, lo, hi: e.tensor_add(
                                    t2v[:, lo * 64:hi * 64],
                                    t1v[:, lo * 64:hi * 64, 0:4],
                                    t1v[:, lo * 64:hi * 64, 4:8]))
                                t3 = big.tile([BLK, NF * 256], BF16, tag="tq3")
                                t3v = t3[:].rearrange("p (s o n) -> p s o n",
                                                      o=2, n=2)
                                split(lambda e, lo, hi: e.tensor_add(
                                    t3v[:, lo * 64:hi * 64],
                                    t2v[:, lo * 64:hi * 64, 0:2],
                                    t2v[:, lo * 64:hi * 64, 2:4]))
                                Av = A[:].rearrange("p (s o n) -> p s o n",
                                                    o=1, n=2)
                                if it == 0:
                                    split(lambda e, lo, hi: e.tensor_add(
                                        Av[:, lo * 64:hi * 64],
                                        t3v[:, lo * 64:hi * 64, 0:1],
                                        t3v[:, lo * 64:hi * 64, 1:2]))
                                else:
                                    agr = work.tile([BLK, NF * 128], BF16,
                                                    tag="agr")
                                    agv = agr[:].rearrange(
                                        "p (s o n) -> p s o n", o=1, n=2)
                                    split(lambda e, lo, hi: e.tensor_add(
                                        agv[:, lo * 64:hi * 64],
                                        t3v[:, lo * 64:hi * 64, 0:1],
                                        t3v[:, lo * 64:hi * 64, 1:2]))
                                    Af = A[:].rearrange("p (f gn) -> p f gn",
                                                        gn=128)
                                    agf = agr[:].rearrange("p (f gn) -> p f gn",
                                                           gn=128)
                                    split(lambda e, lo, hi: e.tensor_add(
                                        Af[:, lo:hi], Af[:, lo:hi],
                                        agf[:, lo:hi]))
                            # v-sum: s = s0 + sum_gi A*U  (tree over gi=64)
                            # op dims (s=(f,gi), o, n2): broadcast of A over o
                            # sits mid-AP, n2 stays contiguous-last -> 2x.
                            Q = big.tile([BLK, NF * 2048], BF16, tag="PQ")
                            Qv = Q[:].rearrange("p (s o n) -> p s o n",
                                                o=16, n=2)
                            Uq = ub[:].rearrange("p (s o n) -> p s o n",
                                                 o=16, n=2)
                            Ab = A[:].rearrange("p (s n) -> p s n", n=2) \
                                .unsqueeze(2).broadcast_to([BLK, NF * 64, 16, 2])
                            split(lambda e, lo, hi: e.tensor_mul(
                                Qv[:, lo * 64:hi * 64], Uq[:, lo * 64:hi * 64],
                                Ab[:, lo * 64:hi * 64]))
                            with nc.allow_low_precision("bf16 tree sums"):
                                Qg = Q[:].rearrange("p (f g s) -> p f g s",
                                                    f=NF, g=64, s=32)
                                q1 = big.tile([BLK, NF * 1024], BF16, tag="tq1")
                                q1v = q1[:].rearrange("p (f g s) -> p f g s",
                                                      f=NF, g=32, s=32)
                                split(lambda e, lo, hi: e.tensor_add(
                                    q1v[:, lo:hi], Qg[:, lo:hi, 0:32],
                                    Qg[:, lo:hi, 32:64]))
                                q2 = big.tile([BLK, NF * 512], BF16, tag="tq2")
                                q2v = q2[:].rearrange("p (f g s) -> p f g s",
                                                      f=NF, g=16, s=32)
                                split(lambda e, lo, hi: e.tensor_add(
                                    q2v[:, lo:hi], q1v[:, lo:hi, 0:16],
                                    q1v[:, lo:hi, 16:32]))
                                q3 = big.tile([BLK, NF * 256], BF16, tag="tq3")
                                q3v = q3[:].rearrange("p (f g s) -> p f g s",
                                                      f=NF, g=8, s=32)
                                split(lambda e, lo, hi: e.tensor_add(
                                    q3v[:, lo:hi], q2v[:, lo:hi, 0:8],
                                    q2v[:, lo:hi, 8:16]))
                                q4 = big.tile([BLK, NF * 128], BF16, tag="q4")
                                q4v = q4[:].rearrange("p (f g s) -> p f g s",
                                                      f=NF, g=4, s=32)
                                split(lambda e, lo, hi: e.tensor_add(
                                    q4v[:, lo:hi], q3v[:, lo:hi, 0:4],
                                    q3v[:, lo:hi, 4:8]))
                                q5 = big.tile([BLK, NF * 64], BF16, tag="q5")
                                q5v = q5[:].rearrange("p (f g s) -> p f g s",
                                                      f=NF, g=2, s=32)
                                split(lambda e, lo, hi: e.tensor_add(
                                    q5v[:, lo:hi], q4v[:, lo:hi, 0:2],
                                    q4v[:, lo:hi, 2:4]))
                                s_blk = work.tile([BLK, 160], F32, tag="s_blk")
                                sv = s_blk[:].rearrange("p (f g s) -> p f g s",
                                                        f=NF, g=1, s=32)
                                split(lambda e, lo, hi: e.tensor_add(
                                    sv[:, lo:hi], q5v[:, lo:hi, 0:1],
                                    q5v[:, lo:hi, 1:2]))
                            # + s0 (same (f, o, n2) layout)
                            nc.vector.tensor_add(s_blk[:], s_blk[:], s0_t[:])
                            # squash (batched over the block's 10 capsules)
                            if it == 0:
                                _squash_on(nc, work, s_blk,
                                           vb16[:].rearrange(
                                               "p (f o n) -> p f o n",
                                               o=16, n=2), lowp=True)
                            else:
                                dst = out_t[:, b * 160:(b + 1) * 160]
                                _squash_on(nc, work, s_blk,
                                           dst.rearrange(
                                               "p (f n o) -> p f o n",
                                               n=2, o=16))
                                nc.sync.dma_start(
                                    out[b * BLK:(b + 1) * BLK, :],
                                    out_t[:, b * 160:(b + 1) * 160])
    nc.compile()
    return nc


def _host_prep(inputs, W):
    """Build per-core input maps from full inputs."""
    import ml_dtypes
    x = np.ascontiguousarray(inputs, dtype=np.float32).reshape(B, R * C, IE)
    Wf = np.ascontiguousarray(W, dtype=np.float32)  # [n, i, e, o]
    # w8[e, (g, i8, f, o, n2)] - compact nonzero block of BD(W)
    w8 = Wf.reshape(NF, 2, NCH, 8, D_IN, CAPS_DIM)  # [f, n2, g, i8, e, o]
    w8 = w8.transpose(4, 2, 3, 0, 5, 1)             # [e, g, i8, f, o, n2]
    w8_a = np.ascontiguousarray(w8).reshape(
        D_IN, NCH * N_CAPS * 128).astype(ml_dtypes.bfloat16)
    # wd[(i8,e), (g, f, o, n2)] - dense W for the s0 chain
    wd = Wf.reshape(NF, 2, N_IN, D_IN, CAPS_DIM)     # [f, n2, i, e, o]
    wd = wd.transpose(2, 3, 0, 4, 1)                 # [i, e, f, o, n2]
    wd = wd.reshape(NCH, 128, N_CAPS * 16)           # [g, (i8 e), 160]
    wd = wd.transpose(1, 0, 2).reshape(128, NCH * N_CAPS * 16)
    wd_a = np.ascontiguousarray(wd).astype(ml_dtypes.bfloat16)
    bpc = B // N_CORES
    in_maps = []
    for c in range(N_CORES):
        xc = x[c * bpc:(c + 1) * bpc].reshape(POS, IE)
        # xT[e, (g, i8, pos)] in bf16
        xt = xc.reshape(POS, N_IN, D_IN).transpose(2, 1, 0)
        xt = np.ascontiguousarray(xt).reshape(D_IN, N_IN * POS)
        # xF[(i8, e), (g, pos)] in bf16 - dense-layout x for the s0 chain
        xf = xc.T.reshape(NCH, 128, POS).transpose(1, 0, 2)
        xf = np.ascontiguousarray(xf).reshape(128, NCH * POS)
        in_maps.append({
            "xT": xt.astype(ml_dtypes.bfloat16),
            "xF": xf.astype(ml_dtypes.bfloat16),
            "w8": w8_a,
            "wd": wd_a,
        })
    return in_maps


_NC_CACHE = []


def kernel(inputs: np.ndarray, W: np.ndarray) -> np.ndarray:
    in_maps = _host_prep(inputs, W)
    if not _NC_CACHE:
        _NC_CACHE.append(build_kernel())
    nc = _NC_CACHE[0]
    res = run_bass_kernel_spmd(nc, in_maps, list(range(N_CORES)))
    outs = [res.results[c]["out"] for c in range(N_CORES)]
    full = np.concatenate(outs, axis=0)  # [3136, 160]
    return full.reshape(B, R, C, N_CAPS, CAPS_DIM)
